# revision 50
# baseline (speedup 1.0000x reference)
"""MergeAdapter (moe_routing) Trainium2 Bass kernel — fp8 DoubleRow version.

Reference computation (per instance n):
    wd = sum_k prob[n,k] * w_down[k]   (D, H)     bd = sum_k prob[n,k] * b_down[k]
    wu = sum_k prob[n,k] * w_up[k]     (H, D)     bu = sum_k prob[n,k] * b_up[k]
    out[n] = x[n] + relu(x[n] @ wd.T + bd) @ wu.T + bu

Sharding: data-parallel over the instance dim N=16 -> 2 instances per core on
8 cores; every core holds the full expert banks. No communication.

Device kernel design (per core):
  - the expert-bank merge runs on the PE as PSUM-accumulated fp8 DoubleRow
    matmuls with host-shipped diagonal tiles: wdm[n] += (p[n,k]*I).T @ wd[k].
    p is split host-side into fp8(p) + fp8(p - fp8(p)) so the routing-weight
    quantization error cancels to ~0.25%; banks ship fp8e4m3. Merged weights
    are evicted from PSUM straight to fp8 tiles (ACT), fp32 accumulation.
  - both adapter matmuls run in fp8e4m3 DoubleRow (0.5 PE cycles/output-row,
    2 contraction k-tiles per instruction): x arrives host-transposed in fp8
    (xT8); relu1 is written in fp8 by the ACT relu eviction with the merged
    b_down as its per-partition bias AP
  - merged b_up rides the second matmul's PSUM group as a K=1 ones-row fp16
    matmul
  - the residual skip-add happens on the store path: an SWDGE accumulate-DMA
    pulls x (fp16, full precision) from DRAM onto the evicted fp16 residual
    tile, then an HWDGE store pushes the finished pair of s-tiles out.
    x is therefore read once (fp8 transposed) plus once on the accumulate
    path (fp16), and no identity matmuls or xn preloads exist at all.
  - PSUM evictions alternate ACT/DVE so neither engine paces the tail.

Schedule: loads run wd-pairs, wu-pairs, xT[0], xT[1] on the sync queue
(consts on the ACT queue so their descriptor generation doesn't lead);
all merges that gate instance 0's store chain are emitted before the
xT-gated mm1[0]; instance 1's wu merge regions and mm1 groups are injected
between instance 0's mm2 s-tiles so the first store never waits on them.

Per-core DMA traffic is 25.2 MB (xT8 4.2 + fp8 banks 4.2 + fp16 x on the
accumulate path 8.4 + fp16 out 8.4), ~71 us of DMA-engine time; TimelineSim
predicts 82.8 us end-to-end and the run is DMA-saturated except a ~3 us
transition dip. Measured 87.3 us best (two paired samples within 0.3%) to
~131 us on hardware depending on the shared terminal's fast/slow epoch,
vs 141.5 us for the previous fp16 baseline in the fast epoch.
Relative error 3.1e-03 (gate 2e-2).

Rejected byte-cut variants (all slower in sim): single fp16 xT read with
on-chip fp8 conversion + B=eye PE transpose skip (91.1 us — the 256 extra
matmuls make the tail PE-bound while tail DMA idles), and the per-instance
hybrid of that with the accumulate path (94.9 us). The accumulate-DMA design
wins because the skip's x bytes ride the tail DMA window that the stores
leave half-empty, costing no engine time at all.
"""
import os
import sys

for _p in ("/opt/trn_rl_repo",):
    if os.path.isdir(_p) and _p not in sys.path:
        sys.path.insert(0, _p)

import ml_dtypes
import numpy as np

import concourse.mybir as mybir
import concourse.tile as tile
from concourse import bacc
from concourse.bass_utils import run_bass_kernel_spmd

N, S, H, K, D = 16, 2048, 1024, 8, 256
NCORES = 8
NPC = N // NCORES          # instances per core
IC = H // 128              # 128-row contraction chunks of the first matmul
IC2 = IC // 2              # DoubleRow 256-row k-tile pairs of mm1
OC = D // 128              # o-chunks (partition tiles of the bottleneck dim)
KP = K // 2                # DoubleRow expert pairs of the merge matmuls
SCW = 512                  # first-matmul psum free-dim width
HCW = 512                  # second-matmul psum free-dim width
MW = 512                   # merge-matmul psum free-dim width (1 psum bank)
ST = S // 128              # s-tiles (output partition tiles)

F16 = mybir.dt.float16
F8 = mybir.dt.float8e4     # e4m3
F32 = mybir.dt.float32
DR = mybir.MatmulPerfMode.DoubleRow
np16 = np.float16
np8 = ml_dtypes.float8_e4m3

_CACHE: dict = {}


def _emit(nc, tc, tens, loop_t=None):
    (xT_d, xn_d, eye_d, wdP_d, wuP_d, eyp_d, bd_d, bu_d, pkn_d, ones_d,
     out_d) = tens
    with (
        tc.tile_pool(name="consts", bufs=1) as consts,
        tc.tile_pool(name="banks", bufs=2 * KP) as banks,
        tc.tile_pool(name="work", bufs=1) as work,
        tc.tile_pool(name="xtp", bufs=1) as xtp,
        tc.tile_pool(name="stream", bufs=6) as stream,
        tc.tile_pool(name="xnp", bufs=ST // 2) as xnp,
        tc.tile_pool(name="psm", bufs=3, space="PSUM") as psm,
        tc.tile_pool(name="pst", bufs=1, space="PSUM") as pstiny,
        tc.tile_pool(name="ps2", bufs=2, space="PSUM") as ps2p,
    ):
        pkn_t = consts.tile([K, NPC], F16, tag="pkn")
        bd_t = consts.tile([K, D], F16, tag="bd")
        bu_t = consts.tile([K, H], F16, tag="bu")
        ones_t = consts.tile([1, 128], F16, tag="ones")
        eye_t = consts.tile([128, 128], F16, tag="eye")
        # eyp: per (n, hi/lo) a [128, K, 128] block-diagonal of p[n,k]*I
        eyp_t = [[consts.tile([128, K, 128], F8, tag=f"eyp{n}{h}",
                              name=f"eyp{n}{h}") for h in range(2)]
                 for n in range(NPC)]
        # consts ride the ACT HWDGE queue so their descriptor generation
        # doesn't serialize ahead of the bank loads on the sync queue
        nc.scalar.dma_start(pkn_t[:], pkn_d.ap())
        nc.scalar.dma_start(bd_t[:], bd_d.ap())
        nc.scalar.dma_start(bu_t[:], bu_d.ap())
        nc.scalar.dma_start(ones_t[:], ones_d.ap())
        nc.scalar.dma_start(eye_t[:], eye_d.ap())
        for n in range(NPC):
            for h in range(2):
                nc.scalar.dma_start(eyp_t[n][h][:], eyp_d.ap()[n, h])

        if loop_t is not None:
            loop_cm = tc.For_i(0, loop_t, 1, hint_engines=tuple(
                getattr(mybir.EngineType, e)
                for e in ("PE", "DVE", "Activation", "SP", "Pool")))
        else:
            import contextlib
            loop_cm = contextlib.nullcontext()

        with loop_cm:
            # ---- merged biases (tiny fp16 matmuls + evictions) ----
            mbd_t = work.tile([128, OC * NPC], F32, tag="mbd")
            mbu_t = [work.tile([1, H], F16, tag=f"mbu{n}", name=f"mbu{n}")
                     for n in range(NPC)]
            for oc in range(OC):
                psbd = pstiny.tile([128, NPC], F32, tag="pst", name="psbd")
                nc.tensor.matmul(psbd[:], bd_t[:, oc * 128:(oc + 1) * 128], pkn_t[:])
                nc.vector.tensor_copy(mbd_t[:, oc * NPC:(oc + 1) * NPC], psbd[:])
            for n in range(NPC):
                for hc in range(H // HCW):
                    psbu = pstiny.tile([1, HCW], F32, tag="pst", name="psbu")
                    nc.tensor.matmul(psbu[:], pkn_t[:, n:n + 1],
                                     bu_t[:, hc * HCW:(hc + 1) * HCW])
                    nc.scalar.copy(mbu_t[n][0:1, hc * HCW:(hc + 1) * HCW], psbu[:])

            # ---- bank-pair + x loads (sync HWDGE), in consumption order:
            # ---- wd pairs, consts, then xt0 slices interleaved with wu
            # ---- pairs (both feed the first store chain), then xt1
            wd_pairs, wu_pairs = [], []
            for j in range(KP):
                bp = banks.tile([128, 2, IC, D], F8, tag="bank", name=f"wdp{j}")
                nc.sync.dma_start(bp[:, 0], wdP_d.ap()[2 * j])
                nc.sync.dma_start(bp[:, 1], wdP_d.ap()[2 * j + 1])
                wd_pairs.append(bp)
            xt = {}
            for n in range(NPC):
                xt[n] = xtp.tile([128, IC, S], F8, tag=f"xt{n}", name=f"xt{n}")
            for j in range(KP):
                bp = banks.tile([128, 2, OC, H], F8, tag="bank", name=f"wup{j}")
                nc.sync.dma_start(bp[:, 0], wuP_d.ap()[2 * j])
                nc.sync.dma_start(bp[:, 1], wuP_d.ap()[2 * j + 1])
                wu_pairs.append(bp)
            for sc in range(S // SCW):
                nc.sync.dma_start(
                    xt[0][:, :, sc * SCW:(sc + 1) * SCW],
                    xT_d.ap()[0][:, :, sc * SCW:(sc + 1) * SCW])
            for sc in range(S // SCW):
                nc.sync.dma_start(
                    xt[1][:, :, sc * SCW:(sc + 1) * SCW],
                    xT_d.ap()[1][:, :, sc * SCW:(sc + 1) * SCW])
            # instance 0's late s-tiles take the eye-matmul skip with x
            # preloaded into the head's post-load DMA window; everything
            # else keeps the accumulate-DMA skip in the tail window
            PRE0 = ST // 2
            xn_pre = {}
            for st in range(PRE0, ST):
                t = xnp.tile([128, H], F16, tag="xn", name=f"xnp{st}")
                nc.sync.dma_start(t[:], xn_d.ap()[0, st * 128:(st + 1) * 128, :])
                xn_pre[st] = t

            # ---- expert merge on the PE: 8 DoubleRow matmuls per 512-col
            # ---- psum region (4 expert pairs x {p_hi, p_lo}), fp32 PSUM,
            # ---- evicted straight to the fp8 merged-weight tiles
            wdm = [work.tile([128, IC, D], F8, tag=f"wdm{n}", name=f"wdm{n}")
                   for n in range(NPC)]
            wum = [work.tile([128, OC, H], F8, tag=f"wum{n}", name=f"wum{n}")
                   for n in range(NPC)]

            def emit_merge_region(n, r, dst_ap, rhs, kind):
                pm = psm.tile(list(dst_ap.shape), F32, tag="psm",
                              name=f"psm_{kind}{n}_{r}")
                for h in range(2):
                    for j in range(KP):
                        nc.tensor.matmul(
                            pm[:],
                            eyp_t[n][h][:, 2 * j:2 * j + 2, :],
                            rhs[j],
                            start=(h == 0 and j == 0),
                            stop=(h == 1 and j == KP - 1),
                            perf_mode=DR)
                ev = nc.scalar.copy if (n + r) % 2 == 0 else nc.vector.tensor_copy
                ev(dst_ap, pm[:])

            def emit_merge(n, regions, kind):
                for r, (dst_ap, rhs) in enumerate(regions):
                    emit_merge_region(n, r, dst_ap, rhs, kind)

            def wd_regions(n):
                # region r covers ic in {2r, 2r+1}: MW contiguous (ic, d) cols
                return [(wdm[n][:, 2 * r:2 * r + 2, :],
                         [bp[:, :, 2 * r:2 * r + 2, :] for bp in wd_pairs])
                        for r in range(IC // 2)]

            def wu_regions(n):
                return [(wum[n][:, r // 2, (r % 2) * HCW:(r % 2 + 1) * HCW],
                         [bp[:, :, r // 2, (r % 2) * HCW:(r % 2 + 1) * HCW]
                          for bp in wu_pairs])
                        for r in range(OC * H // HCW)]


            # ---- matmul 1 (fp8 DoubleRow) + relu(. + bd) -> fp8 relu1 ----
            relu1 = [work.tile([128, OC, S], F8, tag=f"relu{n}", name=f"relu{n}")
                     for n in range(NPC)]

            def mm1_groups(n):
                for sc in range(S // SCW):
                    for oc in range(OC):
                        yield (n, sc, oc)

            def emit_mm1_group(n, sc, oc):
                p1 = psm.tile([128, SCW], F32, tag="psm",
                              name=f"p1_{n}_{sc}_{oc}")
                for ic2 in range(IC2):
                    nc.tensor.matmul(
                        p1[:],
                        wdm[n][:, 2 * ic2:2 * ic2 + 2,
                               oc * 128:(oc + 1) * 128],
                        xt[n][:, 2 * ic2:2 * ic2 + 2,
                              sc * SCW:(sc + 1) * SCW],
                        start=(ic2 == 0), stop=(ic2 == IC2 - 1),
                        perf_mode=DR)
                nc.scalar.activation(
                    relu1[n][:, oc, sc * SCW:(sc + 1) * SCW], p1[:],
                    mybir.ActivationFunctionType.Relu,
                    bias=mbd_t[:, oc * NPC + n:oc * NPC + n + 1],
                    scale=1.0)

            emit_merge(0, wd_regions(0), "wd")
            emit_merge(1, wd_regions(1), "wd")
            emit_merge(0, wu_regions(0), "wu")
            for g in mm1_groups(0):
                emit_mm1_group(*g)
            wu1_rest = [(1, r, dst, rhs, "wu")
                        for r, (dst, rhs) in enumerate(wu_regions(1))]
            mm1_rest = list(mm1_groups(1))

            # ---- matmul 2 (fp8 DoubleRow) + bias; skip-add rides the store
            # ---- path as an SWDGE accumulate-DMA of x onto the residual
            for n in range(NPC):
                ob2 = None
                for st in range(ST):
                    # instance 1's merge regions and mm1 groups slot in
                    # between instance 0's mm2 st-tiles (so the first stores
                    # never wait on them; xt1 arrives while these run)
                    if n == 0 and st >= 1 and wu1_rest:
                        emit_merge_region(*wu1_rest.pop(0))
                    if n == 0 and st >= 2 and mm1_rest:
                        emit_mm1_group(*mm1_rest.pop(0))
                    if st % 2 == 0:
                        ob2 = stream.tile([128, 2, H], F16, tag="ob2", name="ob2")
                    pbig = ps2p.tile([128, H], F32, tag="ps2", name="ps2")
                    for hc in range(H // HCW):
                        nc.tensor.matmul(
                            pbig[:, hc * HCW:(hc + 1) * HCW],
                            relu1[n][:, :, st * 128:(st + 1) * 128],
                            wum[n][:, :, hc * HCW:(hc + 1) * HCW],
                            start=True, stop=False, perf_mode=DR,
                            skip_group_check=True)
                    pre = n == 0 and st >= PRE0
                    for hc in range(H // HCW):
                        nc.tensor.matmul(
                            pbig[:, hc * HCW:(hc + 1) * HCW],
                            ones_t[:], mbu_t[n][0:1, hc * HCW:(hc + 1) * HCW],
                            start=False, stop=not pre, skip_group_check=True)
                    if pre:
                        xn_t = xn_pre[st]
                        for hc in range(H // HCW):
                            nc.tensor.matmul(
                                pbig[:, hc * HCW:(hc + 1) * HCW],
                                eye_t[:], xn_t[:, hc * HCW:(hc + 1) * HCW],
                                start=False, stop=(hc == H // HCW - 1),
                                skip_group_check=True)
                    if st % 2 == 0:
                        nc.scalar.copy(ob2[:, 0, :], pbig[:])
                    else:
                        nc.vector.tensor_copy(ob2[:, 1, :], pbig[:])
                    if st % 2 == 1:
                        dram_pair = (out_d.ap()[n, (st - 1) * 128:(st + 1) * 128, :]
                                     .rearrange("(c p) h -> p c h", p=128))
                        if not pre:
                            nc.gpsimd.dma_start(
                                ob2[:],
                                xn_d.ap()[n, (st - 1) * 128:(st + 1) * 128, :]
                                .rearrange("(c p) h -> p c h", p=128),
                                accum_op=mybir.AluOpType.add)
                        nc.sync.dma_start(dram_pair, ob2[:])


def build(loop_t=None):
    """Build and compile the per-core NEFF. Cached per loop_t."""
    key = loop_t
    if key in _CACHE:
        return _CACHE[key]
    nc = bacc.Bacc("TRN2", target_bir_lowering=False, debug=False,
                   num_devices=NCORES)
    tens = (
        nc.dram_tensor("xT", [NPC, 128, IC, S], F8, kind="ExternalInput"),
        nc.dram_tensor("xn", [NPC, S, H], F16, kind="ExternalInput"),
        nc.dram_tensor("eye", [128, 128], F16, kind="ExternalInput"),
        nc.dram_tensor("wdT", [K, 128, IC, D], F8, kind="ExternalInput"),
        nc.dram_tensor("wuT", [K, 128, OC, H], F8, kind="ExternalInput"),
        nc.dram_tensor("eyp", [NPC, 2, 128, K, 128], F8, kind="ExternalInput"),
        nc.dram_tensor("bd", [K, D], F16, kind="ExternalInput"),
        nc.dram_tensor("bu", [K, H], F16, kind="ExternalInput"),
        nc.dram_tensor("pkn", [K, NPC], F16, kind="ExternalInput"),
        nc.dram_tensor("ones", [1, 128], F16, kind="ExternalInput"),
        nc.dram_tensor("out", [NPC, S, H], F16, kind="ExternalOutput"),
    )
    with tile.TileContext(nc) as tc:
        _emit(nc, tc, tens, loop_t=loop_t)
    nc.compile()
    _CACHE[key] = nc
    return nc


def make_in_maps(hidden_states, prob, w_down, b_down, w_up, b_up):
    """Shard + lay out the full inputs for the 8 cores."""
    hs = np.asarray(hidden_states, dtype=np.float32)
    prob = np.asarray(prob, dtype=np.float32)
    wdT = np.ascontiguousarray(
        np.asarray(w_down, dtype=np.float32).transpose(0, 2, 1)
        .reshape(K, IC, 128, D).transpose(0, 2, 1, 3)).astype(np8)
    wuT = np.ascontiguousarray(
        np.asarray(w_up, dtype=np.float32).transpose(0, 2, 1)
        .reshape(K, OC, 128, H).transpose(0, 2, 1, 3)).astype(np8)
    bd = np.asarray(b_down, dtype=np.float32).astype(np16)
    bu = np.asarray(b_up, dtype=np.float32).astype(np16)
    ones = np.ones((1, 128), dtype=np.float32).astype(np16)
    eye128 = np.eye(128, dtype=np.float32)
    in_maps = []
    for c in range(NCORES):
        shard = hs[c * NPC:(c + 1) * NPC]
        p_shard = prob[c * NPC:(c + 1) * NPC]           # (NPC, K)
        xT = np.ascontiguousarray(
            shard.transpose(0, 2, 1).reshape(NPC, IC, 128, S)
            .transpose(0, 2, 1, 3))
        p_hi = p_shard.astype(np8)
        p_lo = (p_shard - p_hi.astype(np.float32)).astype(np8)
        # eyp[n, h, p, k, m] = (p_hi/p_lo)[n, k] * I[p, m]
        eyp = np.zeros((NPC, 2, 128, K, 128), dtype=np8)
        for n in range(NPC):
            for k in range(K):
                eyp[n, 0, :, k, :] = (eye128 * float(p_hi[n, k])).astype(np8)
                eyp[n, 1, :, k, :] = (eye128 * float(p_lo[n, k])).astype(np8)
        in_maps.append({
            "xT": xT.astype(np8),
            "xn": shard.astype(np16),
            "eye": eye128.astype(np16),
            "wdT": wdT,
            "wuT": wuT,
            "eyp": eyp,
            "bd": bd,
            "bu": bu,
            "pkn": np.ascontiguousarray(p_shard.T).astype(np16),
            "ones": ones,
        })
    return in_maps


def kernel(hidden_states, prob, w_down, b_down, w_up, b_up):
    nc = build()
    in_maps = make_in_maps(hidden_states, prob, w_down, b_down, w_up, b_up)
    res = run_bass_kernel_spmd(nc, in_maps, list(range(NCORES)))
    out = np.concatenate([res.results[c]["out"] for c in range(NCORES)], axis=0)
    return np.ascontiguousarray(out.reshape(N, S, H).astype(np.float32))


# revision 56
# speedup vs baseline: 1.3281x; 1.3281x over previous
"""MergeAdapter (moe_routing) Trainium2 Bass kernel — fp8 DoubleRow version.

Reference computation (per instance n):
    wd = sum_k prob[n,k] * w_down[k]   (D, H)     bd = sum_k prob[n,k] * b_down[k]
    wu = sum_k prob[n,k] * w_up[k]     (H, D)     bu = sum_k prob[n,k] * b_up[k]
    out[n] = x[n] + relu(x[n] @ wd.T + bd) @ wu.T + bu

Sharding: data-parallel over the instance dim N=16 -> 2 instances per core on
8 cores; every core holds the full expert banks. No communication.

Device kernel design (per core):
  - the expert-bank merge runs on the PE as PSUM-accumulated fp8 DoubleRow
    matmuls with host-shipped diagonal tiles: wdm[n] += (p[n,k]*I).T @ wd[k].
    p is split host-side into fp8(p) + fp8(p - fp8(p)) so the routing-weight
    quantization error cancels to ~0.25%; banks ship fp8e4m3. Merged weights
    are evicted from PSUM straight to fp8 tiles (ACT), fp32 accumulation.
  - both adapter matmuls run in fp8e4m3 DoubleRow (0.5 PE cycles/output-row,
    2 contraction k-tiles per instruction): x arrives host-transposed in fp8
    (xT8); relu1 is written in fp8 by the ACT relu eviction with the merged
    b_down as its per-partition bias AP
  - merged b_up rides the second matmul's PSUM group as a K=1 ones-row fp16
    matmul
  - the residual skip-add is phase-balanced: most s-tiles take an SWDGE
    accumulate-DMA that pulls x (fp16, full precision) from DRAM onto the
    evicted residual tile ahead of the HWDGE store; instance 0's last 8
    s-tiles instead preload x in the head's post-load DMA window and add it
    via stationary-eye identity matmuls, so head and tail each move 14.7 MB
    and the bus never idles at the phase transition.
  - PSUM evictions alternate ACT/DVE so neither engine paces the tail.

Schedule: loads run wd-pairs, wu-pairs, xT[0], xT[1] on the sync queue
(consts on the ACT queue so their descriptor generation doesn't lead);
all merges that gate instance 0's store chain are emitted before the
xT-gated mm1[0]; instance 1's wu merge regions and mm1 groups are injected
between instance 0's mm2 s-tiles so the first store never waits on them.

Per-core DMA traffic is 25.2 MB (xT8 4.2 + fp8 banks 4.2 + fp16 x on the
skip path 8.4 + fp16 out 8.4), ~71 us of DMA-engine time; TimelineSim
predicts 80.0 us end-to-end with the bus ~90% busy span-wide — within ~5 us
of the byte floor. Measured 87.3 us (best, on the 82.8 us predecessor) to
~135 us on hardware depending on the shared terminal's fast/slow epoch,
vs 141.5 us for the previous fp16 baseline in the fast epoch.
Relative error 3.1e-03 (gate 2e-2).

Rejected byte-cut variants (all slower in sim): single fp16 xT read with
on-chip fp8 conversion + B=eye PE transpose skip (91.1 us — the 256 extra
matmuls make the tail PE-bound while tail DMA idles), and the per-instance
hybrid of that with the accumulate path (94.9 us). The accumulate-DMA design
wins because the skip's x bytes ride the tail DMA window that the stores
leave half-empty, costing no engine time at all.
"""
import os
import sys

for _p in ("/opt/trn_rl_repo",):
    if os.path.isdir(_p) and _p not in sys.path:
        sys.path.insert(0, _p)

import ml_dtypes
import numpy as np

import concourse.mybir as mybir
import concourse.tile as tile
from concourse import bacc
from concourse.bass_utils import run_bass_kernel_spmd

N, S, H, K, D = 16, 2048, 1024, 8, 256
NCORES = 8
NPC = N // NCORES          # instances per core
IC = H // 128              # 128-row contraction chunks of the first matmul
IC2 = IC // 2              # DoubleRow 256-row k-tile pairs of mm1
OC = D // 128              # o-chunks (partition tiles of the bottleneck dim)
KP = K // 2                # DoubleRow expert pairs of the merge matmuls
SCW = 512                  # first-matmul psum free-dim width
HCW = 512                  # second-matmul psum free-dim width
MW = 512                   # merge-matmul psum free-dim width (1 psum bank)
ST = S // 128              # s-tiles (output partition tiles)

F16 = mybir.dt.float16
F8 = mybir.dt.float8e4     # e4m3
F32 = mybir.dt.float32
DR = mybir.MatmulPerfMode.DoubleRow
np16 = np.float16
np8 = ml_dtypes.float8_e4m3

_CACHE: dict = {}


def _emit(nc, tc, tens, loop_t=None):
    (xT_d, xn_d, eye_d, wdP_d, wuP_d, eyp_d, bd_d, bu_d, pkn_d, ones_d,
     out_d) = tens
    with (
        tc.tile_pool(name="consts", bufs=1) as consts,
        tc.tile_pool(name="banks", bufs=2 * KP) as banks,
        tc.tile_pool(name="work", bufs=1) as work,
        tc.tile_pool(name="xtp", bufs=1) as xtp,
        tc.tile_pool(name="stream", bufs=6) as stream,
        tc.tile_pool(name="xnp", bufs=ST) as xnp,
        tc.tile_pool(name="psm", bufs=3, space="PSUM") as psm,
        tc.tile_pool(name="pst", bufs=1, space="PSUM") as pstiny,
        tc.tile_pool(name="ps2", bufs=2, space="PSUM") as ps2p,
    ):
        pkn_t = consts.tile([K, NPC], F16, tag="pkn")
        bd_t = consts.tile([K, D], F16, tag="bd")
        bu_t = consts.tile([K, H], F16, tag="bu")
        ones_t = consts.tile([1, 128], F16, tag="ones")
        eye_t = consts.tile([128, 128], F16, tag="eye")
        # eyp: per (n, hi/lo) a [128, K, 128] block-diagonal of p[n,k]*I
        eyp_t = [[consts.tile([128, K, 128], F8, tag=f"eyp{n}{h}",
                              name=f"eyp{n}{h}") for h in range(2)]
                 for n in range(NPC)]
        # consts ride the ACT HWDGE queue so their descriptor generation
        # doesn't serialize ahead of the bank loads on the sync queue
        nc.scalar.dma_start(pkn_t[:], pkn_d.ap())
        nc.scalar.dma_start(bd_t[:], bd_d.ap())
        nc.scalar.dma_start(bu_t[:], bu_d.ap())
        nc.scalar.dma_start(ones_t[:], ones_d.ap())
        nc.scalar.dma_start(eye_t[:], eye_d.ap())
        for n in range(NPC):
            for h in range(2):
                nc.scalar.dma_start(eyp_t[n][h][:], eyp_d.ap()[n, h])

        if loop_t is not None:
            loop_cm = tc.For_i(0, loop_t, 1, hint_engines=tuple(
                getattr(mybir.EngineType, e)
                for e in ("PE", "DVE", "Activation", "SP", "Pool")))
        else:
            import contextlib
            loop_cm = contextlib.nullcontext()

        with loop_cm:
            # ---- merged biases (tiny fp16 matmuls + evictions) ----
            mbd_t = work.tile([128, OC * NPC], F32, tag="mbd")
            mbu_t = [work.tile([1, H], F16, tag=f"mbu{n}", name=f"mbu{n}")
                     for n in range(NPC)]
            for oc in range(OC):
                psbd = pstiny.tile([128, NPC], F32, tag="pst", name="psbd")
                nc.tensor.matmul(psbd[:], bd_t[:, oc * 128:(oc + 1) * 128], pkn_t[:])
                nc.vector.tensor_copy(mbd_t[:, oc * NPC:(oc + 1) * NPC], psbd[:])
            for n in range(NPC):
                for hc in range(H // HCW):
                    psbu = pstiny.tile([1, HCW], F32, tag="pst", name="psbu")
                    nc.tensor.matmul(psbu[:], pkn_t[:, n:n + 1],
                                     bu_t[:, hc * HCW:(hc + 1) * HCW])
                    nc.scalar.copy(mbu_t[n][0:1, hc * HCW:(hc + 1) * HCW], psbu[:])

            # ---- bank-pair + x loads (sync HWDGE), in consumption order:
            # ---- wd pairs, consts, then xt0 slices interleaved with wu
            # ---- pairs (both feed the first store chain), then xt1
            wd_pairs, wu_pairs = [], []
            for j in range(KP):
                bp = banks.tile([128, 2, IC, D], F8, tag="bank", name=f"wdp{j}")
                nc.sync.dma_start(bp[:, 0], wdP_d.ap()[2 * j])
                nc.sync.dma_start(bp[:, 1], wdP_d.ap()[2 * j + 1])
                wd_pairs.append(bp)
            xt = {}
            for n in range(NPC):
                xt[n] = xtp.tile([128, IC, S], F8, tag=f"xt{n}", name=f"xt{n}")
            for j in range(KP):
                bp = banks.tile([128, 2, OC, H], F8, tag="bank", name=f"wup{j}")
                nc.sync.dma_start(bp[:, 0], wuP_d.ap()[2 * j])
                nc.sync.dma_start(bp[:, 1], wuP_d.ap()[2 * j + 1])
                wu_pairs.append(bp)
            for sc in range(S // SCW):
                nc.sync.dma_start(
                    xt[0][:, :, sc * SCW:(sc + 1) * SCW],
                    xT_d.ap()[0][:, :, sc * SCW:(sc + 1) * SCW])
            for sc in range(S // SCW):
                nc.sync.dma_start(
                    xt[1][:, :, sc * SCW:(sc + 1) * SCW],
                    xT_d.ap()[1][:, :, sc * SCW:(sc + 1) * SCW])
            # instance 0's late s-tiles take the eye-matmul skip with x
            # preloaded into the head's post-load DMA window; everything
            # else keeps the accumulate-DMA skip in the tail window
            PRE0, PRE1 = 8, 12
            xn_pre = {}
            for n_, st0 in ((0, PRE0), (1, PRE1)):
                for st in range(st0, ST):
                    t = xnp.tile([128, H], F16, tag="xn", name=f"xnp{n_}_{st}")
                    nc.sync.dma_start(
                        t[:], xn_d.ap()[n_, st * 128:(st + 1) * 128, :])
                    xn_pre[(n_, st)] = t

            # ---- expert merge on the PE: 8 DoubleRow matmuls per 512-col
            # ---- psum region (4 expert pairs x {p_hi, p_lo}), fp32 PSUM,
            # ---- evicted straight to the fp8 merged-weight tiles
            wdm = [work.tile([128, IC, D], F8, tag=f"wdm{n}", name=f"wdm{n}")
                   for n in range(NPC)]
            wum = [work.tile([128, OC, H], F8, tag=f"wum{n}", name=f"wum{n}")
                   for n in range(NPC)]

            def emit_merge_region(n, r, dst_ap, rhs, kind):
                pm = psm.tile(list(dst_ap.shape), F32, tag="psm",
                              name=f"psm_{kind}{n}_{r}")
                for h in range(2):
                    for j in range(KP):
                        nc.tensor.matmul(
                            pm[:],
                            eyp_t[n][h][:, 2 * j:2 * j + 2, :],
                            rhs[j],
                            start=(h == 0 and j == 0),
                            stop=(h == 1 and j == KP - 1),
                            perf_mode=DR)
                ev = nc.scalar.copy if (n + r) % 2 == 0 else nc.vector.tensor_copy
                ev(dst_ap, pm[:])

            def emit_merge(n, regions, kind):
                for r, (dst_ap, rhs) in enumerate(regions):
                    emit_merge_region(n, r, dst_ap, rhs, kind)

            def wd_regions(n):
                # region r covers ic in {2r, 2r+1}: MW contiguous (ic, d) cols
                return [(wdm[n][:, 2 * r:2 * r + 2, :],
                         [bp[:, :, 2 * r:2 * r + 2, :] for bp in wd_pairs])
                        for r in range(IC // 2)]

            def wu_regions(n):
                return [(wum[n][:, r // 2, (r % 2) * HCW:(r % 2 + 1) * HCW],
                         [bp[:, :, r // 2, (r % 2) * HCW:(r % 2 + 1) * HCW]
                          for bp in wu_pairs])
                        for r in range(OC * H // HCW)]


            # ---- matmul 1 (fp8 DoubleRow) + relu(. + bd) -> fp8 relu1 ----
            relu1 = [work.tile([128, OC, S], F8, tag=f"relu{n}", name=f"relu{n}")
                     for n in range(NPC)]

            def mm1_groups(n):
                for sc in range(S // SCW):
                    for oc in range(OC):
                        yield (n, sc, oc)

            def emit_mm1_group(n, sc, oc):
                p1 = psm.tile([128, SCW], F32, tag="psm",
                              name=f"p1_{n}_{sc}_{oc}")
                for ic2 in range(IC2):
                    nc.tensor.matmul(
                        p1[:],
                        wdm[n][:, 2 * ic2:2 * ic2 + 2,
                               oc * 128:(oc + 1) * 128],
                        xt[n][:, 2 * ic2:2 * ic2 + 2,
                              sc * SCW:(sc + 1) * SCW],
                        start=(ic2 == 0), stop=(ic2 == IC2 - 1),
                        perf_mode=DR)
                nc.scalar.activation(
                    relu1[n][:, oc, sc * SCW:(sc + 1) * SCW], p1[:],
                    mybir.ActivationFunctionType.Relu,
                    bias=mbd_t[:, oc * NPC + n:oc * NPC + n + 1],
                    scale=1.0)

            emit_merge(0, wd_regions(0), "wd")
            emit_merge(1, wd_regions(1), "wd")
            emit_merge(0, wu_regions(0), "wu")
            for g in mm1_groups(0):
                emit_mm1_group(*g)
            wu1_rest = [(1, r, dst, rhs, "wu")
                        for r, (dst, rhs) in enumerate(wu_regions(1))]
            mm1_rest = list(mm1_groups(1))

            # ---- matmul 2 (fp8 DoubleRow) + bias; skip-add rides the store
            # ---- path as an SWDGE accumulate-DMA of x onto the residual
            for n in range(NPC):
                ob2 = None
                for st in range(ST):
                    # instance 1's merge regions and mm1 groups slot in
                    # between instance 0's mm2 st-tiles (so the first stores
                    # never wait on them; xt1 arrives while these run)
                    if n == 0 and st >= 1 and wu1_rest:
                        emit_merge_region(*wu1_rest.pop(0))
                    if n == 0 and st >= 2 and mm1_rest:
                        emit_mm1_group(*mm1_rest.pop(0))
                    if st % 2 == 0:
                        ob2 = stream.tile([128, 2, H], F16, tag="ob2", name="ob2")
                    pbig = ps2p.tile([128, H], F32, tag="ps2", name="ps2")
                    for hc in range(H // HCW):
                        nc.tensor.matmul(
                            pbig[:, hc * HCW:(hc + 1) * HCW],
                            relu1[n][:, :, st * 128:(st + 1) * 128],
                            wum[n][:, :, hc * HCW:(hc + 1) * HCW],
                            start=True, stop=False, perf_mode=DR,
                            skip_group_check=True)
                    pre = st >= (PRE0 if n == 0 else PRE1)
                    for hc in range(H // HCW):
                        nc.tensor.matmul(
                            pbig[:, hc * HCW:(hc + 1) * HCW],
                            ones_t[:], mbu_t[n][0:1, hc * HCW:(hc + 1) * HCW],
                            start=False, stop=not pre, skip_group_check=True)
                    if pre:
                        xn_t = xn_pre[(n, st)]
                        for hc in range(H // HCW):
                            nc.tensor.matmul(
                                pbig[:, hc * HCW:(hc + 1) * HCW],
                                eye_t[:], xn_t[:, hc * HCW:(hc + 1) * HCW],
                                start=False, stop=(hc == H // HCW - 1),
                                skip_group_check=True)
                    if st % 2 == 0:
                        nc.scalar.copy(ob2[:, 0, :], pbig[:])
                    else:
                        nc.vector.tensor_copy(ob2[:, 1, :], pbig[:])
                    if st % 2 == 1:
                        dram_pair = (out_d.ap()[n, (st - 1) * 128:(st + 1) * 128, :]
                                     .rearrange("(c p) h -> p c h", p=128))
                        if not pre:
                            nc.gpsimd.dma_start(
                                ob2[:],
                                xn_d.ap()[n, (st - 1) * 128:(st + 1) * 128, :]
                                .rearrange("(c p) h -> p c h", p=128),
                                accum_op=mybir.AluOpType.add)
                        nc.sync.dma_start(dram_pair, ob2[:])


def build(loop_t=None):
    """Build and compile the per-core NEFF. Cached per loop_t."""
    key = loop_t
    if key in _CACHE:
        return _CACHE[key]
    nc = bacc.Bacc("TRN2", target_bir_lowering=False, debug=False,
                   num_devices=NCORES)
    tens = (
        nc.dram_tensor("xT", [NPC, 128, IC, S], F8, kind="ExternalInput"),
        nc.dram_tensor("xn", [NPC, S, H], F16, kind="ExternalInput"),
        nc.dram_tensor("eye", [128, 128], F16, kind="ExternalInput"),
        nc.dram_tensor("wdT", [K, 128, IC, D], F8, kind="ExternalInput"),
        nc.dram_tensor("wuT", [K, 128, OC, H], F8, kind="ExternalInput"),
        nc.dram_tensor("eyp", [NPC, 2, 128, K, 128], F8, kind="ExternalInput"),
        nc.dram_tensor("bd", [K, D], F16, kind="ExternalInput"),
        nc.dram_tensor("bu", [K, H], F16, kind="ExternalInput"),
        nc.dram_tensor("pkn", [K, NPC], F16, kind="ExternalInput"),
        nc.dram_tensor("ones", [1, 128], F16, kind="ExternalInput"),
        nc.dram_tensor("out", [NPC, S, H], F16, kind="ExternalOutput"),
    )
    with tile.TileContext(nc) as tc:
        _emit(nc, tc, tens, loop_t=loop_t)
    nc.compile()
    _CACHE[key] = nc
    return nc


def make_in_maps(hidden_states, prob, w_down, b_down, w_up, b_up):
    """Shard + lay out the full inputs for the 8 cores."""
    hs = np.asarray(hidden_states, dtype=np.float32)
    prob = np.asarray(prob, dtype=np.float32)
    wdT = np.ascontiguousarray(
        np.asarray(w_down, dtype=np.float32).transpose(0, 2, 1)
        .reshape(K, IC, 128, D).transpose(0, 2, 1, 3)).astype(np8)
    wuT = np.ascontiguousarray(
        np.asarray(w_up, dtype=np.float32).transpose(0, 2, 1)
        .reshape(K, OC, 128, H).transpose(0, 2, 1, 3)).astype(np8)
    bd = np.asarray(b_down, dtype=np.float32).astype(np16)
    bu = np.asarray(b_up, dtype=np.float32).astype(np16)
    ones = np.ones((1, 128), dtype=np.float32).astype(np16)
    eye128 = np.eye(128, dtype=np.float32)
    in_maps = []
    for c in range(NCORES):
        shard = hs[c * NPC:(c + 1) * NPC]
        p_shard = prob[c * NPC:(c + 1) * NPC]           # (NPC, K)
        xT = np.ascontiguousarray(
            shard.transpose(0, 2, 1).reshape(NPC, IC, 128, S)
            .transpose(0, 2, 1, 3))
        p_hi = p_shard.astype(np8)
        p_lo = (p_shard - p_hi.astype(np.float32)).astype(np8)
        # eyp[n, h, p, k, m] = (p_hi/p_lo)[n, k] * I[p, m]
        eyp = np.zeros((NPC, 2, 128, K, 128), dtype=np8)
        for n in range(NPC):
            for k in range(K):
                eyp[n, 0, :, k, :] = (eye128 * float(p_hi[n, k])).astype(np8)
                eyp[n, 1, :, k, :] = (eye128 * float(p_lo[n, k])).astype(np8)
        in_maps.append({
            "xT": xT.astype(np8),
            "xn": shard.astype(np16),
            "eye": eye128.astype(np16),
            "wdT": wdT,
            "wuT": wuT,
            "eyp": eyp,
            "bd": bd,
            "bu": bu,
            "pkn": np.ascontiguousarray(p_shard.T).astype(np16),
            "ones": ones,
        })
    return in_maps


def kernel(hidden_states, prob, w_down, b_down, w_up, b_up):
    nc = build()
    in_maps = make_in_maps(hidden_states, prob, w_down, b_down, w_up, b_up)
    res = run_bass_kernel_spmd(nc, in_maps, list(range(NCORES)))
    out = np.concatenate([res.results[c]["out"] for c in range(NCORES)], axis=0)
    return np.ascontiguousarray(out.reshape(N, S, H).astype(np.float32))
